# revision 7
# baseline (speedup 1.0000x reference)
"""Trainium2 Bass kernel for nn_AttentionBlock (sliding-window GQA attention block).

Sharding: sequence-parallel over 8 cores. Core c owns query rows
[c*512, (c+1)*512) and recomputes K/V for the 3 aligned 512-row blocks
[(c-2)*512, (c+1)*512) that its 1024-wide causal window can touch
(out-of-range blocks are zero-padded and masked).

Per-core pipeline (all matmuls fp16 operands, fp32 PSUM accumulate):
  1. K/V projections -> RMS stats -> RoPE (norm weights folded into host
     RoPE tables; K's rstd folded into the softmax exp scale) -> PE
     transpose K to [dk, seq] layout; V kept [seq, dk].
  2. Q projection -> RMS/RoPE -> *rstd -> PE transpose to [dk, seq].
  3. Attention per head: scores computed transposed S^T[k, q] so that
     P^T tiles feed the PV matmul directly (lhsT = V). Softmax without
     max-subtraction (scores bounded ~5); denominator via ones-matmul;
     normalization applied to O^T with a gpsimd partition-broadcast of
     the reciprocal.
  4. Output projection from the transposed attention output, streamed
     against the (host-pre-transposed) Wo.
"""

import os
import sys

import numpy as np

for _p in ("/opt/trn_rl_repo",):
    if _p not in sys.path and os.path.isdir(_p):
        sys.path.insert(0, _p)

import concourse.bass as bass
import concourse.mybir as mybir
import concourse.tile as tile
from concourse import bacc
from concourse.bass_utils import run_bass_kernel_spmd
from concourse.masks import make_identity

F16 = mybir.dt.float16
F32 = mybir.dt.float32

N_CORES = 8
S, D = 4096, 2048
H, KV, DK = 16, 4, 128
GSZ = H // KV  # heads per kv group
WINDOW = 1024
THETA = 500000.0
EPS = 1e-6

SQ = S // N_CORES          # 512 query rows per core
NQT = SQ // 128            # 4 query chunks
NKT = 12                   # 12 kv chunks of 128 (3 blocks of 512)
SKV = NKT * 128            # 1536
NE = D // 128              # 16 contraction chunks
NDT = D // 512             # 4 tiles of 512 along output dims


def _broadcast_free(ap, count, axis):
    """Insert a 0-step (broadcast) free dim of length `count` at `axis`
    (free-dim index, 0-based after the partition dim)."""
    new = list(ap.ap)
    new.insert(1 + axis, [0, count])
    return bass.AP(tensor=ap.tensor, offset=ap.offset, ap=new)


def _rope_pairs(ap):
    """View a [128, n*128] AP as ([128, n, 64] even, [128, n, 64] odd)."""
    r = ap.rearrange("p (h m two) -> p h m two", two=2, m=64)
    return r[:, :, :, 0], r[:, :, :, 1]


def _emit_rope(nc, pool, src, dst, tabs, nheads, cast_scalars=None):
    """dst[:, h*128+d] = rope(src) using tables tabs = (cosA, sinA, sinB, cosB)
    each a [128, 64] AP broadcast across the nheads dim.

    If cast_scalars is given, it is a list of nheads [128,1] APs; the final
    per-head result is written as dst_head = tmp_head * scalar (fused cast).
    Otherwise results are written directly to dst.
    """
    ev, od = _rope_pairs(src)
    cosA, sinA, sinB, cosB = (_broadcast_free(t, nheads, 0) for t in tabs)
    if cast_scalars is None:
        out_ev, out_od = _rope_pairs(dst)
        tmp_ev, tmp_od = out_ev, out_od
        tmp = None
    else:
        tmp = pool.tile([128, nheads * 128], F32, tag="rope_tmp")
        tmp_ev, tmp_od = _rope_pairs(tmp)
    t1 = pool.tile([128, nheads, 64], F32, tag="rope_t1")
    t2 = pool.tile([128, nheads, 64], F32, tag="rope_t2")
    nc.vector.tensor_mul(t1, ev, cosA)
    nc.vector.tensor_mul(t2, od, sinA)
    nc.vector.tensor_sub(tmp_ev, t1, t2)
    t3 = pool.tile([128, nheads, 64], F32, tag="rope_t1")
    t4 = pool.tile([128, nheads, 64], F32, tag="rope_t2")
    nc.vector.tensor_mul(t3, ev, sinB)
    nc.vector.tensor_mul(t4, od, cosB)
    nc.vector.tensor_add(tmp_od, t3, t4)
    if cast_scalars is not None:
        for hh in range(nheads):
            nc.vector.tensor_scalar_mul(
                dst[:, hh * 128:(hh + 1) * 128],
                tmp[:, hh * 128:(hh + 1) * 128],
                cast_scalars[hh],
            )


def _rms_stats(nc, pool, src, sqrt_bias, sqrt_scale, out_recip):
    """out_recip[128,1] = 1/sqrt(sum(src^2)*sqrt_scale + sqrt_bias) for a
    [128, 128] src slice. (ACT Square w/ accumulate, ACT Sqrt, DVE recip.)"""
    scr = pool.tile([128, 128], F32, tag="rms_scr")
    ssq = pool.tile([128, 1], F32, tag="rms_ssq")
    nc.scalar.activation(out=scr, in_=src, func=mybir.ActivationFunctionType.Square,
                         accum_out=ssq)
    srt = pool.tile([128, 1], F32, tag="rms_srt")
    nc.scalar.activation(out=srt, in_=ssq, func=mybir.ActivationFunctionType.Sqrt,
                         bias=sqrt_bias, scale=sqrt_scale)
    nc.vector.reciprocal(out=out_recip, in_=srt)


def build_program():
    nc = bacc.Bacc("TRN2", target_bir_lowering=False, debug=False)

    xq_t = nc.declare_dram_parameter("xq_t", [128, NE, SQ], F16, isOutput=False)
    xk_t = nc.declare_dram_parameter("xk_t", [3, 128, NE, 512], F16, isOutput=False)
    xv_t = nc.declare_dram_parameter("xv_t", [3, 128, NE, 512], F16, isOutput=False)
    wq_t = nc.declare_dram_parameter("wq_t", [NDT, 128, NE, 512], F16, isOutput=False)
    wk_t = nc.declare_dram_parameter("wk_t", [128, NE, 512], F16, isOutput=False)
    wv_t = nc.declare_dram_parameter("wv_t", [128, NE, 512], F16, isOutput=False)
    wo_t = nc.declare_dram_parameter("wo_t", [NDT, 128, NE, 512], F16, isOutput=False)
    ropeq = nc.declare_dram_parameter("ropeq", [128, 4, NQT, 64], F32, isOutput=False)
    ropek = nc.declare_dram_parameter("ropek", [128, 4, NKT, 64], F32, isOutput=False)
    pmask = nc.declare_dram_parameter("pmask", [128, NQT, 9, 128], F16, isOutput=False)
    y = nc.declare_dram_parameter("y", [SQ, D], F32, isOutput=True)

    EXP = mybir.ActivationFunctionType.Exp

    with tile.TileContext(nc) as tc:
        with tc.tile_pool(name="const", bufs=1) as const, \
             tc.tile_pool(name="persist", bufs=1) as persist:
            ident = const.tile([128, 128], F16)
            make_identity(nc, ident)
            ones_t = const.tile([128, 1], F16)
            nc.vector.memset(ones_t, 1.0)
            bias_k = const.tile([128, 1], F32)
            nc.vector.memset(bias_k, 128.0 * EPS)
            bias_q = const.tile([128, 1], F32)
            nc.vector.memset(bias_q, EPS)
            masks = const.tile([128, NQT, 9, 128], F16)
            nc.sync.dma_start(out=masks, in_=pmask[:, :, :, :])
            rq_sb = const.tile([128, 4, NQT, 64], F32)
            nc.sync.dma_start(out=rq_sb, in_=ropeq[:, :, :, :])
            rk_sb = const.tile([128, 4, NKT, 64], F32)
            nc.sync.dma_start(out=rk_sb, in_=ropek[:, :, :, :])

            kT = persist.tile([128, KV, NKT, 128], F16)
            vt = persist.tile([128, NKT, KV, 128], F16)
            qT = persist.tile([128, H, SQ], F16)
            aoT = persist.tile([128, H, SQ], F16)
            rstdk = persist.tile([128, NKT, KV], F32)

            # ---------------- K/V phase ----------------
            with tc.tile_pool(name="kv_w", bufs=1) as kvw, \
                 tc.tile_pool(name="kv_stage", bufs=2) as kvs, \
                 tc.tile_pool(name="kv_sb", bufs=3) as kvsb, \
                 tc.tile_pool(name="kv_ps", bufs=2, space="PSUM") as kvps:
                wk_sb = kvw.tile([128, NE, 512], F16)
                nc.sync.dma_start(out=wk_sb, in_=wk_t[:, :, :])
                wv_sb = kvw.tile([128, NE, 512], F16)
                nc.sync.dma_start(out=wv_sb, in_=wv_t[:, :, :])
                for b in range(3):
                    xk_sb = kvs.tile([128, NE, 512], F16, tag="xk")
                    nc.sync.dma_start(out=xk_sb, in_=xk_t[b])
                    xv_sb = kvs.tile([128, NE, 512], F16, tag="xv")
                    nc.sync.dma_start(out=xv_sb, in_=xv_t[b])
                    for sc in range(4):
                        kc = b * 4 + sc
                        ssl = slice(sc * 128, (sc + 1) * 128)
                        k_ps = kvps.tile([128, 512], F32, tag="kps")
                        for ec in range(NE):
                            nc.tensor.matmul(k_ps, xk_sb[:, ec, ssl], wk_sb[:, ec, :],
                                             start=(ec == 0), stop=(ec == NE - 1))
                        for g in range(KV):
                            _rms_stats(nc, kvsb, k_ps[:, g * 128:(g + 1) * 128],
                                       sqrt_bias=bias_k, sqrt_scale=1.0,
                                       out_recip=rstdk[:, kc, g:g + 1])
                        krot = kvsb.tile([128, 512], F16, tag="krot")
                        tabs = tuple(rk_sb[:, t, kc, :] for t in range(4))
                        _emit_rope(nc, kvsb, k_ps[:, :], krot[:, :], tabs, KV)
                        for g in range(KV):
                            ktp = kvps.tile([128, 128], F16, tag="ktp")
                            nc.tensor.transpose(ktp, krot[:, g * 128:(g + 1) * 128], ident)
                            nc.vector.tensor_copy(out=kT[:, g, kc, :], in_=ktp)
                        v_ps = kvps.tile([128, 512], F32, tag="vps")
                        for ec in range(NE):
                            nc.tensor.matmul(v_ps, xv_sb[:, ec, ssl], wv_sb[:, ec, :],
                                             start=(ec == 0), stop=(ec == NE - 1))
                        nc.vector.tensor_copy(
                            out=vt[:, kc, :, :],
                            in_=v_ps.rearrange("p (g d) -> p g d", g=KV))

            # ---------------- Q phase ----------------
            with tc.tile_pool(name="q_stage", bufs=1) as qs, \
                 tc.tile_pool(name="q_w", bufs=2) as qw, \
                 tc.tile_pool(name="q_sb", bufs=3) as qsb, \
                 tc.tile_pool(name="q_ps", bufs=3, space="PSUM") as qps, \
                 tc.tile_pool(name="q_tps", bufs=2, space="PSUM") as qtps:
                xq_sb = qs.tile([128, NE, SQ], F16)
                nc.sync.dma_start(out=xq_sb, in_=xq_t[:, :, :])
                for dt in range(NDT):
                    wq_sb = qw.tile([128, NE, 512], F16, tag="wq")
                    nc.sync.dma_start(out=wq_sb, in_=wq_t[dt])
                    for sc in range(NQT):
                        ssl = slice(sc * 128, (sc + 1) * 128)
                        q_ps = qps.tile([128, 512], F32, tag="qps")
                        for ec in range(NE):
                            nc.tensor.matmul(q_ps, xq_sb[:, ec, ssl], wq_sb[:, ec, :],
                                             start=(ec == 0), stop=(ec == NE - 1))
                        rstd_q = []
                        for hh in range(4):
                            r = qsb.tile([128, 1], F32, tag="rstdq")
                            _rms_stats(nc, qsb, q_ps[:, hh * 128:(hh + 1) * 128],
                                       sqrt_bias=bias_q, sqrt_scale=1.0 / 128.0,
                                       out_recip=r)
                            rstd_q.append(r)
                        qrot = qsb.tile([128, 512], F16, tag="qrot")
                        tabs = tuple(rq_sb[:, t, sc, :] for t in range(4))
                        _emit_rope(nc, qsb, q_ps[:, :], qrot[:, :], tabs, 4,
                                   cast_scalars=rstd_q)
                        for hh in range(4):
                            h = dt * 4 + hh
                            qtp = qtps.tile([128, 128], F16, tag="qtp")
                            nc.tensor.transpose(qtp, qrot[:, hh * 128:(hh + 1) * 128],
                                                ident)
                            nc.vector.tensor_copy(out=qT[:, h, ssl], in_=qtp)

            # ---------------- attention phase ----------------
            with tc.tile_pool(name="p_pool", bufs=3) as pp, \
                 tc.tile_pool(name="a_sb", bufs=2) as asb, \
                 tc.tile_pool(name="a_sc", bufs=3, space="PSUM") as asc, \
                 tc.tile_pool(name="a_oc", bufs=2, space="PSUM") as aoc, \
                 tc.tile_pool(name="a_dn", bufs=2, space="PSUM") as adn:
                for g in range(KV):
                    for hh in range(GSZ):
                        h = g * GSZ + hh
                        P_h = pp.tile([128, NKT, 512], F16, tag="P")
                        for kc in range(NKT):
                            qb_lo, qb_hi = max(0, kc - 8), min(NQT - 1, kc)
                            qsl = slice(qb_lo * 128, (qb_hi + 1) * 128)
                            s_ps = asc.tile([128, 512], F32, tag="score")
                            nc.tensor.matmul(s_ps[:, qsl], kT[:, g, kc, :],
                                             qT[:, h, qsl], start=True, stop=True)
                            nc.scalar.activation(out=P_h[:, kc, qsl], in_=s_ps[:, qsl],
                                                 func=EXP, scale=rstdk[:, kc, g:g + 1])
                            for qb in range(qb_lo, qb_hi + 1):
                                qbs = slice(qb * 128, (qb + 1) * 128)
                                nc.vector.tensor_mul(P_h[:, kc, qbs], P_h[:, kc, qbs],
                                                     masks[:, qb, kc - qb, :])
                        den_ps = adn.tile([1, 512], F32, tag="den")
                        for qb in range(NQT):
                            qbs = slice(qb * 128, (qb + 1) * 128)
                            for j in range(9):
                                nc.tensor.matmul(den_ps[0:1, qbs], ones_t,
                                                 P_h[:, qb + j, qbs],
                                                 start=(j == 0), stop=(j == 8))
                        o_ps = aoc.tile([128, 512], F32, tag="oacc")
                        for qb in range(NQT):
                            qbs = slice(qb * 128, (qb + 1) * 128)
                            for j in range(9):
                                nc.tensor.matmul(o_ps[:, qbs], vt[:, qb + j, g, :],
                                                 P_h[:, qb + j, qbs],
                                                 start=(j == 0), stop=(j == 8))
                        den_sb = asb.tile([1, 512], F32, tag="den_sb")
                        nc.vector.reciprocal(out=den_sb, in_=den_ps)
                        recb = asb.tile([128, 512], F32, tag="recb")
                        nc.gpsimd.partition_broadcast(recb, den_sb[0:1, :])
                        nc.vector.tensor_mul(aoT[:, h, :], o_ps, recb)

            # ---------------- output projection ----------------
            with tc.tile_pool(name="o_w", bufs=2) as ow, \
                 tc.tile_pool(name="o_sb", bufs=3) as osb, \
                 tc.tile_pool(name="o_ps", bufs=3, space="PSUM") as ops:
                for ot in range(NDT):
                    wo_sb = ow.tile([128, NE, 512], F16, tag="wo")
                    nc.sync.dma_start(out=wo_sb, in_=wo_t[ot])
                    for sc in range(NQT):
                        ssl = slice(sc * 128, (sc + 1) * 128)
                        y_ps = ops.tile([128, 512], F32, tag="yacc")
                        for dc in range(NE):
                            nc.tensor.matmul(y_ps, aoT[:, dc, ssl], wo_sb[:, dc, :],
                                             start=(dc == 0), stop=(dc == NE - 1))
                        y_sb = osb.tile([128, 512], F32, tag="ysb")
                        nc.vector.tensor_copy(out=y_sb, in_=y_ps)
                        nc.sync.dma_start(
                            out=y[sc * 128:(sc + 1) * 128, ot * 512:(ot + 1) * 512],
                            in_=y_sb)

    nc.compile()
    return nc


# ---------------- host-side packing ----------------

def _tile_emajor(a16, col0, ncols):
    """[2048, N] (e-major) f16 array -> [128, 16, ncols] tiled view."""
    sl = a16[:, col0:col0 + ncols]
    return np.ascontiguousarray(sl.reshape(NE, 128, ncols).transpose(1, 0, 2))


def _rope_tables(pos, norm_w):
    """-> [128, 4, nchunks, 64] f32 tables (cosA, sinA, sinB, cosB) with the
    per-dim norm weights folded in. pos: [n*128] positions."""
    freqs = 1.0 / (THETA ** (np.arange(0, DK, 2, dtype=np.float64) / DK))
    ang = np.outer(pos.astype(np.float64), freqs)
    cos = np.cos(ang).astype(np.float32)
    sin = np.sin(ang).astype(np.float32)
    w_ev = norm_w[0::2].astype(np.float32)
    w_od = norm_w[1::2].astype(np.float32)
    tabs = np.stack([cos * w_ev, sin * w_od, sin * w_ev, cos * w_od])  # [4, n*128, 64]
    n = pos.shape[0] // 128
    return np.ascontiguousarray(
        tabs.reshape(4, n, 128, 64).transpose(2, 0, 1, 3))


def _masks_for_core(c):
    out = np.zeros((128, NQT, 9, 128), np.float16)
    p = np.arange(128)
    q = np.arange(128)
    for qb in range(NQT):
        for j in range(9):
            kchunk = c * 4 - 8 + qb + j
            iglob = c * SQ + qb * 128 + q[None, :]
            jglob = kchunk * 128 + p[:, None]
            ok = (jglob >= 0) & (iglob - jglob >= 0) & (iglob - jglob < WINDOW)
            out[:, qb, j, :] = ok.astype(np.float16)
    return out


_PROGRAM = None


def _get_program():
    global _PROGRAM
    if _PROGRAM is None:
        _PROGRAM = build_program()
    return _PROGRAM


def _pack_in_maps(xq, xk, xv, Wq, Wk, Wv, Wo, q_norm_w, k_norm_w):
    xqT = np.ascontiguousarray(np.asarray(xq, np.float32)[0].T).astype(np.float16)
    xkT = np.asarray(xk, np.float32)[0].T.astype(np.float16)
    xvT = np.asarray(xv, np.float32)[0].T.astype(np.float16)
    pad = np.zeros((D, 2 * SQ), np.float16)
    xkTp = np.concatenate([pad, xkT], axis=1)  # col i = global row i - 1024
    xvTp = np.concatenate([pad, xvT], axis=1)

    wq16 = np.ascontiguousarray(np.asarray(Wq, np.float32).T).astype(np.float16)
    wk16 = np.ascontiguousarray(np.asarray(Wk, np.float32).T).astype(np.float16)
    wv16 = np.ascontiguousarray(np.asarray(Wv, np.float32).T).astype(np.float16)
    wo16 = np.ascontiguousarray(np.asarray(Wo, np.float32).T).astype(np.float16)

    wq_t = np.stack([_tile_emajor(wq16, dt * 512, 512) for dt in range(NDT)])
    wk_t = _tile_emajor(wk16, 0, 512)
    wv_t = _tile_emajor(wv16, 0, 512)
    wo_t = np.stack([_tile_emajor(wo16, ot * 512, 512) for ot in range(NDT)])

    qw = np.asarray(q_norm_w, np.float32)
    kw = np.asarray(k_norm_w, np.float32)

    in_maps = []
    for c in range(N_CORES):
        xq_t = _tile_emajor(xqT, c * SQ, SQ)
        xk_tc = np.stack([_tile_emajor(xkTp, (c + b) * 512, 512) for b in range(3)])
        xv_tc = np.stack([_tile_emajor(xvTp, (c + b) * 512, 512) for b in range(3)])
        qpos = c * SQ + np.arange(SQ)
        kpos = (c - 2) * 512 + np.arange(SKV)
        in_maps.append({
            "xq_t": xq_t, "xk_t": xk_tc, "xv_t": xv_tc,
            "wq_t": wq_t, "wk_t": wk_t, "wv_t": wv_t, "wo_t": wo_t,
            "ropeq": _rope_tables(qpos, qw),
            "ropek": _rope_tables(kpos, kw),
            "pmask": _masks_for_core(c),
        })
    return in_maps


def kernel(xq, xk, xv, Wq, Wk, Wv, Wo, q_norm_w, k_norm_w):
    nc = _get_program()
    in_maps = _pack_in_maps(xq, xk, xv, Wq, Wk, Wv, Wo, q_norm_w, k_norm_w)
    res = run_bass_kernel_spmd(nc, in_maps, core_ids=list(range(N_CORES)))
    out = np.concatenate([res.results[c]["y"] for c in range(N_CORES)], axis=0)
    return out.reshape(1, S, D).astype(np.float32)


def kernel_with_results(trace=False, tmpdir=None, **inputs):
    """Devloop entry: same as kernel() but also returns the raw
    BassKernelResults (exec_time_ns etc. when trace is enabled)."""
    nc = _get_program()
    in_maps = _pack_in_maps(**inputs)
    res = run_bass_kernel_spmd(nc, in_maps, core_ids=list(range(N_CORES)),
                               trace=trace, tmpdir=tmpdir)
    out = np.concatenate([res.results[c]["y"] for c in range(N_CORES)], axis=0)
    return out.reshape(1, S, D).astype(np.float32), res


# revision 18
# speedup vs baseline: 1.1216x; 1.1216x over previous
"""Trainium2 Bass kernel for nn_AttentionBlock (sliding-window GQA attention block).

Sharding: sequence-parallel over 8 cores. Core c owns query rows
[c*512, (c+1)*512) and recomputes K/V for the 3 aligned 512-row blocks
[(c-2)*512, (c+1)*512) that its 1024-wide causal window can touch
(out-of-range blocks are zero-padded and masked).

Per-core pipeline (all matmuls fp16 operands, fp32 PSUM accumulate):
  1. K/V projections -> RMS stats -> RoPE (norm weights folded into host
     RoPE tables; K's rstd folded into the softmax exp scale) -> PE
     transpose K to [dk, seq] layout; V kept [seq, dk].
  2. Q projection -> RMS/RoPE -> *rstd -> PE transpose to [dk, seq].
  3. Attention per head: scores computed transposed S^T[k, q] so that
     P^T tiles feed the PV matmul directly (lhsT = V). Softmax without
     max-subtraction (scores bounded ~5); denominator via ones-matmul;
     normalization applied to O^T with a gpsimd partition-broadcast of
     the reciprocal.
  4. Output projection from the transposed attention output, streamed
     against the (host-pre-transposed) Wo.
"""

import os
import sys

import numpy as np

for _p in ("/opt/trn_rl_repo",):
    if _p not in sys.path and os.path.isdir(_p):
        sys.path.insert(0, _p)

import concourse.bass as bass
import concourse.mybir as mybir
import concourse.tile as tile
from concourse import bacc
from concourse.bass_utils import run_bass_kernel_spmd
from concourse.masks import make_identity

F16 = mybir.dt.float16
F32 = mybir.dt.float32

N_CORES = 8
S, D = 4096, 2048
H, KV, DK = 16, 4, 128
GSZ = H // KV  # heads per kv group
WINDOW = 1024
THETA = 500000.0
EPS = 1e-6

SQ = S // N_CORES          # 512 query rows per core
NQT = SQ // 128            # 4 query chunks
NKT = 12                   # 12 kv chunks of 128 (3 blocks of 512)
SKV = NKT * 128            # 1536
NE = D // 128              # 16 contraction chunks
NDT = D // 512             # 4 tiles of 512 along output dims


def _broadcast_free(ap, count, axis):
    """Insert a 0-step (broadcast) free dim of length `count` at `axis`
    (free-dim index, 0-based after the partition dim)."""
    new = list(ap.ap)
    new.insert(1 + axis, [0, count])
    return bass.AP(tensor=ap.tensor, offset=ap.offset, ap=new)


def _rope_pairs(ap):
    """View a [128, n*128] AP as ([128, n, 64] even, [128, n, 64] odd)."""
    r = ap.rearrange("p (h m two) -> p h m two", two=2, m=64)
    return r[:, :, :, 0], r[:, :, :, 1]


def _emit_rope(nc, pool, src, dst, tabs, nheads, cast_scalars=None):
    """dst[:, h*128+d] = rope(src) using tables tabs = (cosA, sinA, sinB, cosB)
    each a [128, 64] AP broadcast across the nheads dim.

    If cast_scalars is given, it is a list of nheads [128,1] APs; the final
    per-head result is written as dst_head = tmp_head * scalar (fused cast).
    Otherwise results are written directly to dst.
    """
    ev, od = _rope_pairs(src)
    cosA, sinA, sinB, cosB = (_broadcast_free(t, nheads, 0) for t in tabs)
    if cast_scalars is None:
        out_ev, out_od = _rope_pairs(dst)
        tmp_ev, tmp_od = out_ev, out_od
        tmp = None
    else:
        tmp = pool.tile([128, nheads * 128], F32, tag="rope_tmp")
        tmp_ev, tmp_od = _rope_pairs(tmp)
    t1 = pool.tile([128, nheads, 64], F32, tag="rope_t1")
    t2 = pool.tile([128, nheads, 64], F32, tag="rope_t2")
    nc.vector.tensor_mul(t1, ev, cosA)
    nc.vector.tensor_mul(t2, od, sinA)
    nc.vector.tensor_sub(tmp_ev, t1, t2)
    t3 = pool.tile([128, nheads, 64], F32, tag="rope_t1")
    t4 = pool.tile([128, nheads, 64], F32, tag="rope_t2")
    nc.vector.tensor_mul(t3, ev, sinB)
    nc.vector.tensor_mul(t4, od, cosB)
    nc.vector.tensor_add(tmp_od, t3, t4)
    if cast_scalars is not None:
        for hh in range(nheads):
            nc.vector.tensor_scalar_mul(
                dst[:, hh * 128:(hh + 1) * 128],
                tmp[:, hh * 128:(hh + 1) * 128],
                cast_scalars[hh],
            )


def _rms_stats4(nc, pool, src, sqrt_bias, sqrt_scale, out_recip4):
    """out_recip4[128,4] = 1/sqrt(sum(head_sq)*sqrt_scale + sqrt_bias) for the
    four 128-wide head slices of a [128, 512] src tile."""
    ssq4 = pool.tile([128, 4], F32, tag="rms_ssq4")
    for hh in range(4):
        scr = pool.tile([128, 128], F32, tag="rms_scr")
        nc.scalar.activation(out=scr, in_=src[:, hh * 128:(hh + 1) * 128],
                             func=mybir.ActivationFunctionType.Square,
                             accum_out=ssq4[:, hh:hh + 1])
    srt4 = pool.tile([128, 4], F32, tag="rms_srt4")
    nc.scalar.activation(out=srt4, in_=ssq4, func=mybir.ActivationFunctionType.Sqrt,
                         bias=sqrt_bias, scale=sqrt_scale)
    nc.vector.reciprocal(out=out_recip4, in_=srt4)


def build_program():
    nc = bacc.Bacc("TRN2", target_bir_lowering=False, debug=False)

    xq_t = nc.declare_dram_parameter("xq_t", [128, NE, SQ], F16, isOutput=False)
    xk_t = nc.declare_dram_parameter("xk_t", [3, 128, NE, 512], F16, isOutput=False)
    xv_t = nc.declare_dram_parameter("xv_t", [3, 128, NE, 512], F16, isOutput=False)
    wq_t = nc.declare_dram_parameter("wq_t", [NDT, 128, NE, 512], F16, isOutput=False)
    wk_t = nc.declare_dram_parameter("wk_t", [128, NE, 512], F16, isOutput=False)
    wv_t = nc.declare_dram_parameter("wv_t", [128, NE, 512], F16, isOutput=False)
    wo_t = nc.declare_dram_parameter("wo_t", [NDT, 128, NE, 512], F16, isOutput=False)
    ropeq = nc.declare_dram_parameter("ropeq", [128, 4, NQT, 64], F32, isOutput=False)
    ropek = nc.declare_dram_parameter("ropek", [128, 4, NKT, 64], F32, isOutput=False)
    pmask = nc.declare_dram_parameter("pmask", [128, NQT, 2, 128], F16, isOutput=False)
    padrow = nc.declare_dram_parameter("padrow", [1, SQ], F32, isOutput=False)
    y = nc.declare_dram_parameter("y", [SQ, D], F32, isOutput=True)

    EXP = mybir.ActivationFunctionType.Exp

    with tile.TileContext(nc) as tc:
        with tc.tile_pool(name="const", bufs=1) as const, \
             tc.tile_pool(name="persist", bufs=1) as persist:
            ident = const.tile([128, 128], F16)
            make_identity(nc, ident)
            ones_t = const.tile([128, 1], F16)
            nc.vector.memset(ones_t, 1.0)
            bias_k = const.tile([128, 1], F32)
            nc.vector.memset(bias_k, 128.0 * EPS)
            bias_q = const.tile([128, 1], F32)
            nc.vector.memset(bias_q, EPS)
            # constants ride the ACT HWDGE ring so they don't delay the
            # K/V weight+activation loads on the Sync ring at startup
            masks = const.tile([128, NQT, 2, 128], F16)
            nc.scalar.dma_start(out=masks, in_=pmask[:, :, :, :])
            rq_sb = const.tile([128, 4, NQT, 64], F32)
            nc.scalar.dma_start(out=rq_sb, in_=ropeq[:, :, :, :])
            rk_sb = const.tile([128, 4, NKT, 64], F32)
            nc.scalar.dma_start(out=rk_sb, in_=ropek[:, :, :, :])
            padrow_sb = const.tile([1, SQ], F32)
            nc.scalar.dma_start(out=padrow_sb, in_=padrow[:, :])

            kT = persist.tile([128, KV, NKT, 128], F16)
            vt = persist.tile([128, NKT, KV, 128], F16)
            qT = persist.tile([128, H, SQ], F16)
            aoT = persist.tile([128, H, SQ], F16)
            rstdk = persist.tile([128, NKT, KV], F32)

            # ---------------- K/V phase ----------------
            with tc.tile_pool(name="kv_w", bufs=1) as kvw, \
                 tc.tile_pool(name="kv_stage", bufs=2) as kvs, \
                 tc.tile_pool(name="kv_sb", bufs=3) as kvsb, \
                 tc.tile_pool(name="kv_ps", bufs=2, space="PSUM") as kvps:
                wk_sb = kvw.tile([128, NE, 512], F16)
                nc.sync.dma_start(out=wk_sb, in_=wk_t[:, :, :])
                wv_sb = kvw.tile([128, NE, 512], F16)
                nc.sync.dma_start(out=wv_sb, in_=wv_t[:, :, :])
                for b in range(3):
                    xk_sb = kvs.tile([128, NE, 512], F16, tag="xk")
                    nc.sync.dma_start(out=xk_sb, in_=xk_t[b])
                    xv_sb = kvs.tile([128, NE, 512], F16, tag="xv")
                    nc.sync.dma_start(out=xv_sb, in_=xv_t[b])
                    for sc in range(4):
                        kc = b * 4 + sc
                        ssl = slice(sc * 128, (sc + 1) * 128)
                        k_ps = kvps.tile([128, 512], F32, tag="kps")
                        for ec in range(NE):
                            nc.tensor.matmul(k_ps, xk_sb[:, ec, ssl], wk_sb[:, ec, :],
                                             start=(ec == 0), stop=(ec == NE - 1))
                        _rms_stats4(nc, kvsb, k_ps, sqrt_bias=bias_k,
                                    sqrt_scale=1.0, out_recip4=rstdk[:, kc, :])
                        krot = kvsb.tile([128, 512], F16, tag="krot")
                        tabs = tuple(rk_sb[:, t, kc, :] for t in range(4))
                        _emit_rope(nc, kvsb, k_ps[:, :], krot[:, :], tabs, KV)
                        for g in range(KV):
                            ktp = kvps.tile([128, 128], F16, tag="ktp")
                            nc.tensor.transpose(ktp, krot[:, g * 128:(g + 1) * 128], ident)
                            nc.vector.tensor_copy(out=kT[:, g, kc, :], in_=ktp)
                        v_ps = kvps.tile([128, 512], F32, tag="vps")
                        for ec in range(NE):
                            nc.tensor.matmul(v_ps, xv_sb[:, ec, ssl], wv_sb[:, ec, :],
                                             start=(ec == 0), stop=(ec == NE - 1))
                        nc.vector.tensor_copy(
                            out=vt[:, kc, :, :],
                            in_=v_ps.rearrange("p (g d) -> p g d", g=KV))

            # ---------------- Q phase ----------------
            with tc.tile_pool(name="q_stage", bufs=1) as qs, \
                 tc.tile_pool(name="q_w", bufs=2) as qw, \
                 tc.tile_pool(name="q_sb", bufs=3) as qsb, \
                 tc.tile_pool(name="q_ps", bufs=3, space="PSUM") as qps, \
                 tc.tile_pool(name="q_tps", bufs=2, space="PSUM") as qtps:
                xq_sb = qs.tile([128, NE, SQ], F16)
                nc.scalar.dma_start(out=xq_sb, in_=xq_t[:, :, :])
                for dt in range(NDT):
                    wq_sb = qw.tile([128, NE, 512], F16, tag="wq")
                    nc.sync.dma_start(out=wq_sb, in_=wq_t[dt])
                    for sc in range(NQT):
                        ssl = slice(sc * 128, (sc + 1) * 128)
                        q_ps = qps.tile([128, 512], F32, tag="qps")
                        for ec in range(NE):
                            nc.tensor.matmul(q_ps, xq_sb[:, ec, ssl], wq_sb[:, ec, :],
                                             start=(ec == 0), stop=(ec == NE - 1))
                        rq4 = qsb.tile([128, 4], F32, tag="rstdq4")
                        _rms_stats4(nc, qsb, q_ps, sqrt_bias=bias_q,
                                    sqrt_scale=1.0 / 128.0, out_recip4=rq4)
                        rstd_q = [rq4[:, hh:hh + 1] for hh in range(4)]
                        qrot = qsb.tile([128, 512], F16, tag="qrot")
                        tabs = tuple(rq_sb[:, t, sc, :] for t in range(4))
                        _emit_rope(nc, qsb, q_ps[:, :], qrot[:, :], tabs, 4,
                                   cast_scalars=rstd_q)
                        for hh in range(4):
                            h = dt * 4 + hh
                            qtp = qtps.tile([128, 128], F16, tag="qtp")
                            nc.tensor.transpose(qtp, qrot[:, hh * 128:(hh + 1) * 128],
                                                ident)
                            nc.vector.tensor_copy(out=qT[:, h, ssl], in_=qtp)

            # ---------------- attention phase ----------------
            import concourse.bass_isa as bass_isa
            with tc.tile_pool(name="p_pool", bufs=3) as pp, \
                 tc.tile_pool(name="a_sb", bufs=2) as asb, \
                 tc.tile_pool(name="a_sc", bufs=3, space="PSUM") as asc, \
                 tc.tile_pool(name="a_oc", bufs=2, space="PSUM") as aoc, \
                 tc.tile_pool(name="a_dn", bufs=2, space="PSUM") as adn:
                for g in range(KV):
                    for hh in range(GSZ):
                        h = g * GSZ + hh
                        P_h = pp.tile([128, NKT, 512], F16, tag="P")
                        for kc in range(NKT):
                            qb_lo, qb_hi = max(0, kc - 8), min(NQT - 1, kc)
                            qsl = slice(qb_lo * 128, (qb_hi + 1) * 128)
                            s_ps = asc.tile([128, 512], F32, tag="score")
                            nc.tensor.matmul(s_ps[:, qsl], kT[:, g, kc, :],
                                             qT[:, h, qsl], start=True, stop=True)
                            nc.scalar.activation(out=P_h[:, kc, qsl], in_=s_ps[:, qsl],
                                                 func=EXP, scale=rstdk[:, kc, g:g + 1])
                        # corner masks only: (qb, kc=qb) diag-causal and
                        # (qb, kc=qb+8) window edge, all 4 qb in one strided op.
                        # P_h free layout: kc*512 + q; diag tiles at qb*640,
                        # edge tiles at qb*640 + 8*512. masks: [p, qb, 2, 128].
                        pfull = P_h[:, 0, 0:128]
                        mfull = masks[:, 0, 0, :]
                        for jj, pbase in ((0, 0), (1, 8 * 512)):
                            pap = bass.AP(tensor=pfull.tensor,
                                          offset=pfull.offset + pbase,
                                          ap=[pfull.ap[0], [640, NQT], [1, 128]])
                            map_ = bass.AP(tensor=mfull.tensor,
                                           offset=mfull.offset + jj * 128,
                                           ap=[mfull.ap[0], [256, NQT], [1, 128]])
                            nc.vector.tensor_mul(pap, pap, map_)
                        # denominator: ones-matmul accumulation per query block
                        # (pad keys on interior tiles contribute exp(0)=1 each;
                        # corrected below via the host-computed padrow).
                        den_ps = adn.tile([1, 512], F32, tag="den")
                        for qb in range(NQT):
                            qbs = slice(qb * 128, (qb + 1) * 128)
                            for j in range(9):
                                nc.tensor.matmul(den_ps[0:1, qbs], ones_t,
                                                 P_h[:, qb + j, qbs],
                                                 start=(j == 0), stop=(j == 8))
                        o_ps = aoc.tile([128, 512], F32, tag="oacc")
                        for qb in range(NQT):
                            qbs = slice(qb * 128, (qb + 1) * 128)
                            for j in range(9):
                                nc.tensor.matmul(o_ps[:, qbs], vt[:, qb + j, g, :],
                                                 P_h[:, qb + j, qbs],
                                                 start=(j == 0), stop=(j == 8))
                        den_sb = asb.tile([1, 512], F32, tag="den_sb")
                        nc.vector.scalar_tensor_tensor(
                            out=den_sb, in0=den_ps[0:1, :], scalar=1.0,
                            in1=padrow_sb[0:1, :],
                            op0=mybir.AluOpType.mult,
                            op1=mybir.AluOpType.subtract)
                        den_rc = asb.tile([1, 512], F32, tag="den_rc")
                        nc.vector.reciprocal(out=den_rc, in_=den_sb)
                        recb = asb.tile([128, 512], F32, tag="recb")
                        nc.gpsimd.partition_broadcast(recb, den_rc[0:1, :])
                        nc.vector.tensor_mul(aoT[:, h, :], o_ps, recb)

            # ---------------- output projection ----------------
            with tc.tile_pool(name="o_w", bufs=2) as ow, \
                 tc.tile_pool(name="o_sb", bufs=3) as osb, \
                 tc.tile_pool(name="o_ps", bufs=3, space="PSUM") as ops:
                for ot in range(NDT):
                    wo_sb = ow.tile([128, NE, 512], F16, tag="wo")
                    nc.sync.dma_start(out=wo_sb, in_=wo_t[ot])
                    for sc in range(NQT):
                        ssl = slice(sc * 128, (sc + 1) * 128)
                        y_ps = ops.tile([128, 512], F32, tag="yacc")
                        for dc in range(NE):
                            nc.tensor.matmul(y_ps, aoT[:, dc, ssl], wo_sb[:, dc, :],
                                             start=(dc == 0), stop=(dc == NE - 1))
                        y_sb = osb.tile([128, 512], F32, tag="ysb")
                        nc.vector.tensor_copy(out=y_sb, in_=y_ps)
                        nc.sync.dma_start(
                            out=y[sc * 128:(sc + 1) * 128, ot * 512:(ot + 1) * 512],
                            in_=y_sb)

    nc.compile()
    return nc


# ---------------- host-side packing ----------------

def _tile_emajor(a16, col0, ncols):
    """[2048, N] (e-major) f16 array -> [128, 16, ncols] tiled view."""
    sl = a16[:, col0:col0 + ncols]
    return np.ascontiguousarray(sl.reshape(NE, 128, ncols).transpose(1, 0, 2))


def _rope_tables(pos, norm_w):
    """-> [128, 4, nchunks, 64] f32 tables (cosA, sinA, sinB, cosB) with the
    per-dim norm weights folded in. pos: [n*128] positions."""
    freqs = 1.0 / (THETA ** (np.arange(0, DK, 2, dtype=np.float64) / DK))
    ang = np.outer(pos.astype(np.float64), freqs)
    cos = np.cos(ang).astype(np.float32)
    sin = np.sin(ang).astype(np.float32)
    w_ev = norm_w[0::2].astype(np.float32)
    w_od = norm_w[1::2].astype(np.float32)
    tabs = np.stack([cos * w_ev, sin * w_od, sin * w_ev, cos * w_od])  # [4, n*128, 64]
    n = pos.shape[0] // 128
    return np.ascontiguousarray(
        tabs.reshape(4, n, 128, 64).transpose(2, 0, 1, 3))


def _masks_for_core(c):
    """Corner masks only: jj=0 -> kc=qb (window edge, lag 8);
    jj=1 -> kc=qb+8 (causal diagonal, lag 0)."""
    out = np.zeros((128, NQT, 2, 128), np.float16)
    p = np.arange(128)
    q = np.arange(128)
    for qb in range(NQT):
        for jj, j in ((0, 0), (1, 8)):
            kchunk = c * 4 - 8 + qb + j
            iglob = c * SQ + qb * 128 + q[None, :]
            jglob = kchunk * 128 + p[:, None]
            ok = (jglob >= 0) & (iglob - jglob >= 0) & (iglob - jglob < WINDOW)
            out[:, qb, jj, :] = ok.astype(np.float16)
    return out


def _padrow_for_core(c):
    """Per-q-position count of zero-padded keys that land on interior
    (unmasked) window tiles: exp(0)=1 each, subtracted from the softmax
    denominator. Interior tiles are kc=qb+1..qb+7; tile kc is fully padded
    iff global chunk c*4-8+qb+j < 0."""
    out = np.zeros((1, SQ), np.float32)
    for qb in range(NQT):
        npad = int(np.clip(7 - 4 * c - qb, 0, 7))
        out[0, qb * 128:(qb + 1) * 128] = 128.0 * npad
    return out


_PROGRAM = None


def _get_program():
    global _PROGRAM
    if _PROGRAM is None:
        _PROGRAM = build_program()
    return _PROGRAM


def _pack_in_maps(xq, xk, xv, Wq, Wk, Wv, Wo, q_norm_w, k_norm_w):
    xqT = np.ascontiguousarray(np.asarray(xq, np.float32)[0].T).astype(np.float16)
    xkT = np.asarray(xk, np.float32)[0].T.astype(np.float16)
    xvT = np.asarray(xv, np.float32)[0].T.astype(np.float16)
    pad = np.zeros((D, 2 * SQ), np.float16)
    xkTp = np.concatenate([pad, xkT], axis=1)  # col i = global row i - 1024
    xvTp = np.concatenate([pad, xvT], axis=1)

    wq16 = np.ascontiguousarray(np.asarray(Wq, np.float32).T).astype(np.float16)
    wk16 = np.ascontiguousarray(np.asarray(Wk, np.float32).T).astype(np.float16)
    wv16 = np.ascontiguousarray(np.asarray(Wv, np.float32).T).astype(np.float16)
    wo16 = np.ascontiguousarray(np.asarray(Wo, np.float32).T).astype(np.float16)

    wq_t = np.stack([_tile_emajor(wq16, dt * 512, 512) for dt in range(NDT)])
    wk_t = _tile_emajor(wk16, 0, 512)
    wv_t = _tile_emajor(wv16, 0, 512)
    wo_t = np.stack([_tile_emajor(wo16, ot * 512, 512) for ot in range(NDT)])

    qw = np.asarray(q_norm_w, np.float32)
    kw = np.asarray(k_norm_w, np.float32)

    in_maps = []
    for c in range(N_CORES):
        xq_t = _tile_emajor(xqT, c * SQ, SQ)
        xk_tc = np.stack([_tile_emajor(xkTp, (c + b) * 512, 512) for b in range(3)])
        xv_tc = np.stack([_tile_emajor(xvTp, (c + b) * 512, 512) for b in range(3)])
        qpos = c * SQ + np.arange(SQ)
        kpos = (c - 2) * 512 + np.arange(SKV)
        in_maps.append({
            "xq_t": xq_t, "xk_t": xk_tc, "xv_t": xv_tc,
            "wq_t": wq_t, "wk_t": wk_t, "wv_t": wv_t, "wo_t": wo_t,
            "ropeq": _rope_tables(qpos, qw),
            "ropek": _rope_tables(kpos, kw),
            "pmask": _masks_for_core(c),
            "padrow": _padrow_for_core(c),
        })
    return in_maps


def kernel(xq, xk, xv, Wq, Wk, Wv, Wo, q_norm_w, k_norm_w):
    nc = _get_program()
    in_maps = _pack_in_maps(xq, xk, xv, Wq, Wk, Wv, Wo, q_norm_w, k_norm_w)
    res = run_bass_kernel_spmd(nc, in_maps, core_ids=list(range(N_CORES)))
    out = np.concatenate([res.results[c]["y"] for c in range(N_CORES)], axis=0)
    return out.reshape(1, S, D).astype(np.float32)


def kernel_with_results(trace=False, tmpdir=None, **inputs):
    """Devloop entry: same as kernel() but also returns the raw
    BassKernelResults (exec_time_ns etc. when trace is enabled)."""
    nc = _get_program()
    in_maps = _pack_in_maps(**inputs)
    res = run_bass_kernel_spmd(nc, in_maps, core_ids=list(range(N_CORES)),
                               trace=trace, tmpdir=tmpdir)
    out = np.concatenate([res.results[c]["y"] for c in range(N_CORES)], axis=0)
    return out.reshape(1, S, D).astype(np.float32), res


# revision 21
# speedup vs baseline: 1.1319x; 1.0092x over previous
"""Trainium2 Bass kernel for nn_AttentionBlock (sliding-window GQA attention block).

Sharding: sequence-parallel over 8 cores. Core c owns query rows
[c*512, (c+1)*512) and recomputes K/V for the 3 aligned 512-row blocks
[(c-2)*512, (c+1)*512) that its 1024-wide causal window can touch
(out-of-range blocks are zero-padded and masked).

Per-core pipeline (all matmuls fp16 operands, fp32 PSUM accumulate):
  1. K/V projections -> RMS stats -> RoPE (norm weights folded into host
     RoPE tables; K's rstd folded into the softmax exp scale) -> PE
     transpose K to [dk, seq] layout; V kept [seq, dk].
  2. Q projection -> RMS/RoPE -> *rstd -> PE transpose to [dk, seq].
  3. Attention per head: scores computed transposed S^T[k, q] so that
     P^T tiles feed the PV matmul directly (lhsT = V). Softmax without
     max-subtraction (scores bounded ~5); denominator via ones-matmul;
     normalization applied to O^T with a gpsimd partition-broadcast of
     the reciprocal.
  4. Output projection from the transposed attention output, streamed
     against the (host-pre-transposed) Wo.
"""

import os
import sys

import numpy as np

for _p in ("/opt/trn_rl_repo",):
    if _p not in sys.path and os.path.isdir(_p):
        sys.path.insert(0, _p)

import concourse.bass as bass
import concourse.mybir as mybir
import concourse.tile as tile
from concourse import bacc
from concourse.bass_utils import run_bass_kernel_spmd
from concourse.masks import make_identity

F16 = mybir.dt.float16
F32 = mybir.dt.float32

N_CORES = 8
S, D = 4096, 2048
H, KV, DK = 16, 4, 128
GSZ = H // KV  # heads per kv group
WINDOW = 1024
THETA = 500000.0
EPS = 1e-6

SQ = S // N_CORES          # 512 query rows per core
NQT = SQ // 128            # 4 query chunks
NKT = 12                   # 12 kv chunks of 128 (3 blocks of 512)
SKV = NKT * 128            # 1536
NE = D // 128              # 16 contraction chunks
NDT = D // 512             # 4 tiles of 512 along output dims


def _broadcast_free(ap, count, axis):
    """Insert a 0-step (broadcast) free dim of length `count` at `axis`
    (free-dim index, 0-based after the partition dim)."""
    new = list(ap.ap)
    new.insert(1 + axis, [0, count])
    return bass.AP(tensor=ap.tensor, offset=ap.offset, ap=new)


def _rope_pairs(ap):
    """View a [128, n*128] AP as ([128, n, 64] even, [128, n, 64] odd)."""
    r = ap.rearrange("p (h m two) -> p h m two", two=2, m=64)
    return r[:, :, :, 0], r[:, :, :, 1]


def _emit_rope(nc, pool, src, dst, tabs, nheads, cast_scalars=None):
    """dst[:, h*128+d] = rope(src) using tables tabs = (cosA, sinA, sinB, cosB)
    each a [128, 64] AP broadcast across the nheads dim.

    If cast_scalars is given, it is a list of nheads [128,1] APs; the final
    per-head result is written as dst_head = tmp_head * scalar (fused cast).
    Otherwise results are written directly to dst.
    """
    ev, od = _rope_pairs(src)
    cosA, sinA, sinB, cosB = (_broadcast_free(t, nheads, 0) for t in tabs)
    if cast_scalars is None:
        out_ev, out_od = _rope_pairs(dst)
        tmp_ev, tmp_od = out_ev, out_od
        tmp = None
    else:
        tmp = pool.tile([128, nheads * 128], F32, tag="rope_tmp")
        tmp_ev, tmp_od = _rope_pairs(tmp)
    t1 = pool.tile([128, nheads, 64], F32, tag="rope_t1")
    t2 = pool.tile([128, nheads, 64], F32, tag="rope_t2")
    nc.vector.tensor_mul(t1, ev, cosA)
    nc.vector.tensor_mul(t2, od, sinA)
    nc.vector.tensor_sub(tmp_ev, t1, t2)
    t3 = pool.tile([128, nheads, 64], F32, tag="rope_t1")
    t4 = pool.tile([128, nheads, 64], F32, tag="rope_t2")
    nc.vector.tensor_mul(t3, ev, sinB)
    nc.vector.tensor_mul(t4, od, cosB)
    nc.vector.tensor_add(tmp_od, t3, t4)
    if cast_scalars is not None:
        for hh in range(nheads):
            nc.vector.tensor_scalar_mul(
                dst[:, hh * 128:(hh + 1) * 128],
                tmp[:, hh * 128:(hh + 1) * 128],
                cast_scalars[hh],
            )


def _rms_stats4(nc, pool, src, sqrt_bias, sqrt_scale, out_recip4):
    """out_recip4[128,4] = 1/sqrt(sum(head_sq)*sqrt_scale + sqrt_bias) for the
    four 128-wide head slices of a [128, 512] src tile."""
    ssq4 = pool.tile([128, 4], F32, tag="rms_ssq4")
    for hh in range(4):
        scr = pool.tile([128, 128], F32, tag="rms_scr")
        nc.scalar.activation(out=scr, in_=src[:, hh * 128:(hh + 1) * 128],
                             func=mybir.ActivationFunctionType.Square,
                             accum_out=ssq4[:, hh:hh + 1])
    srt4 = pool.tile([128, 4], F32, tag="rms_srt4")
    nc.scalar.activation(out=srt4, in_=ssq4, func=mybir.ActivationFunctionType.Sqrt,
                         bias=sqrt_bias, scale=sqrt_scale)
    nc.vector.reciprocal(out=out_recip4, in_=srt4)


def build_program():
    nc = bacc.Bacc("TRN2", target_bir_lowering=False, debug=False)

    xq_t = nc.declare_dram_parameter("xq_t", [128, NE, SQ], F16, isOutput=False)
    xk_t = nc.declare_dram_parameter("xk_t", [3, 128, NE, 512], F16, isOutput=False)
    xv_t = nc.declare_dram_parameter("xv_t", [3, 128, NE, 512], F16, isOutput=False)
    wq_t = nc.declare_dram_parameter("wq_t", [NDT, 128, NE, 512], F16, isOutput=False)
    wk_t = nc.declare_dram_parameter("wk_t", [128, NE, 512], F16, isOutput=False)
    wv_t = nc.declare_dram_parameter("wv_t", [128, NE, 512], F16, isOutput=False)
    wo_t = nc.declare_dram_parameter("wo_t", [NDT, 128, NE, 512], F16, isOutput=False)
    ropeq = nc.declare_dram_parameter("ropeq", [128, 4, NQT, 64], F32, isOutput=False)
    ropek = nc.declare_dram_parameter("ropek", [128, 4, NKT, 64], F32, isOutput=False)
    pmask = nc.declare_dram_parameter("pmask", [128, NQT, 2, 128], F16, isOutput=False)
    padrow = nc.declare_dram_parameter("padrow", [1, SQ], F32, isOutput=False)
    y = nc.declare_dram_parameter("y", [SQ, D], F32, isOutput=True)

    EXP = mybir.ActivationFunctionType.Exp

    with tile.TileContext(nc) as tc:
        with tc.tile_pool(name="const", bufs=1) as const, \
             tc.tile_pool(name="persist", bufs=1) as persist:
            ident = const.tile([128, 128], F16)
            make_identity(nc, ident)
            ones_t = const.tile([128, 1], F16)
            nc.vector.memset(ones_t, 1.0)
            bias_k = const.tile([128, 1], F32)
            nc.vector.memset(bias_k, 128.0 * EPS)
            bias_q = const.tile([128, 1], F32)
            nc.vector.memset(bias_q, EPS)
            # constants ride the ACT HWDGE ring so they don't delay the
            # K/V weight+activation loads on the Sync ring at startup
            masks = const.tile([128, NQT, 2, 128], F16)
            nc.scalar.dma_start(out=masks, in_=pmask[:, :, :, :])
            rq_sb = const.tile([128, 4, NQT, 64], F32)
            nc.scalar.dma_start(out=rq_sb, in_=ropeq[:, :, :, :])
            rk_sb = const.tile([128, 4, NKT, 64], F32)
            nc.scalar.dma_start(out=rk_sb, in_=ropek[:, :, :, :])
            padrow_sb = const.tile([1, SQ], F32)
            nc.scalar.dma_start(out=padrow_sb, in_=padrow[:, :])

            kT = persist.tile([128, KV, NKT, 128], F16)
            vt = persist.tile([128, NKT, KV, 128], F16)
            qT = persist.tile([128, H, SQ], F16)
            aoT = persist.tile([128, H, SQ], F16)
            rstdk = persist.tile([128, NKT, KV], F32)

            # ---------------- K/V phase ----------------
            # activations/weights staged as 4-ec-chunk tiles so the first
            # matmuls start after ~1MB rather than after the full 2MB load,
            # and chunk DMAs pipeline against the accumulation.
            with tc.tile_pool(name="kv_w", bufs=1) as kvw, \
                 tc.tile_pool(name="kv_stage", bufs=2) as kvs, \
                 tc.tile_pool(name="kv_sb", bufs=3) as kvsb, \
                 tc.tile_pool(name="kv_ps", bufs=3, space="PSUM") as kvps, \
                 tc.tile_pool(name="kv_tp", bufs=2, space="PSUM") as kvtp:
                wk_c, wv_c = [], []
                for i in range(4):
                    t = kvw.tile([128, 4, 512], F16, tag=f"wk{i}")
                    nc.sync.dma_start(out=t, in_=wk_t[:, 4 * i:4 * i + 4, :])
                    wk_c.append(t)
                    t = kvw.tile([128, 4, 512], F16, tag=f"wv{i}")
                    nc.sync.dma_start(out=t, in_=wv_t[:, 4 * i:4 * i + 4, :])
                    wv_c.append(t)
                for b in range(3):
                    xk_c, xv_c = [], []
                    for i in range(4):
                        t = kvs.tile([128, 4, 512], F16, tag=f"xk{i}")
                        nc.sync.dma_start(out=t, in_=xk_t[b][:, 4 * i:4 * i + 4, :])
                        xk_c.append(t)
                        t = kvs.tile([128, 4, 512], F16, tag=f"xv{i}")
                        nc.sync.dma_start(out=t, in_=xv_t[b][:, 4 * i:4 * i + 4, :])
                        xv_c.append(t)
                    for sc in range(4):
                        kc = b * 4 + sc
                        ssl = slice(sc * 128, (sc + 1) * 128)
                        k_ps = kvps.tile([128, 512], F32, tag="kps")
                        for ec in range(NE):
                            nc.tensor.matmul(k_ps, xk_c[ec // 4][:, ec % 4, ssl],
                                             wk_c[ec // 4][:, ec % 4, :],
                                             start=(ec == 0), stop=(ec == NE - 1))
                        _rms_stats4(nc, kvsb, k_ps, sqrt_bias=bias_k,
                                    sqrt_scale=1.0, out_recip4=rstdk[:, kc, :])
                        krot = kvsb.tile([128, 512], F16, tag="krot")
                        tabs = tuple(rk_sb[:, t, kc, :] for t in range(4))
                        _emit_rope(nc, kvsb, k_ps[:, :], krot[:, :], tabs, KV)
                        for g in range(KV):
                            ktp = kvtp.tile([128, 128], F16, tag="ktp")
                            nc.tensor.transpose(ktp, krot[:, g * 128:(g + 1) * 128], ident)
                            nc.vector.tensor_copy(out=kT[:, g, kc, :], in_=ktp)
                        v_ps = kvps.tile([128, 512], F32, tag="vps")
                        for ec in range(NE):
                            nc.tensor.matmul(v_ps, xv_c[ec // 4][:, ec % 4, ssl],
                                             wv_c[ec // 4][:, ec % 4, :],
                                             start=(ec == 0), stop=(ec == NE - 1))
                        nc.vector.tensor_copy(
                            out=vt[:, kc, :, :],
                            in_=v_ps.rearrange("p (g d) -> p g d", g=KV))

            # ---------------- Q phase ----------------
            with tc.tile_pool(name="q_stage", bufs=1) as qs, \
                 tc.tile_pool(name="q_w", bufs=2) as qw, \
                 tc.tile_pool(name="q_sb", bufs=3) as qsb, \
                 tc.tile_pool(name="q_ps", bufs=3, space="PSUM") as qps, \
                 tc.tile_pool(name="q_tps", bufs=2, space="PSUM") as qtps:
                xq_sb = qs.tile([128, NE, SQ], F16)
                nc.scalar.dma_start(out=xq_sb, in_=xq_t[:, :, :])
                for dt in range(NDT):
                    wq_c = []
                    for i in range(4):
                        t = qw.tile([128, 4, 512], F16, tag=f"wq{i}")
                        nc.sync.dma_start(out=t, in_=wq_t[dt][:, 4 * i:4 * i + 4, :])
                        wq_c.append(t)
                    for sc in range(NQT):
                        ssl = slice(sc * 128, (sc + 1) * 128)
                        q_ps = qps.tile([128, 512], F32, tag="qps")
                        for ec in range(NE):
                            nc.tensor.matmul(q_ps, xq_sb[:, ec, ssl],
                                             wq_c[ec // 4][:, ec % 4, :],
                                             start=(ec == 0), stop=(ec == NE - 1))
                        rq4 = qsb.tile([128, 4], F32, tag="rstdq4")
                        _rms_stats4(nc, qsb, q_ps, sqrt_bias=bias_q,
                                    sqrt_scale=1.0 / 128.0, out_recip4=rq4)
                        rstd_q = [rq4[:, hh:hh + 1] for hh in range(4)]
                        qrot = qsb.tile([128, 512], F16, tag="qrot")
                        tabs = tuple(rq_sb[:, t, sc, :] for t in range(4))
                        _emit_rope(nc, qsb, q_ps[:, :], qrot[:, :], tabs, 4,
                                   cast_scalars=rstd_q)
                        for hh in range(4):
                            h = dt * 4 + hh
                            qtp = qtps.tile([128, 128], F16, tag="qtp")
                            nc.tensor.transpose(qtp, qrot[:, hh * 128:(hh + 1) * 128],
                                                ident)
                            nc.vector.tensor_copy(out=qT[:, h, ssl], in_=qtp)

            # ---------------- attention phase ----------------
            import concourse.bass_isa as bass_isa
            with tc.tile_pool(name="p_pool", bufs=3) as pp, \
                 tc.tile_pool(name="a_sb", bufs=2) as asb, \
                 tc.tile_pool(name="a_sc", bufs=3, space="PSUM") as asc, \
                 tc.tile_pool(name="a_oc", bufs=2, space="PSUM") as aoc, \
                 tc.tile_pool(name="a_dn", bufs=2, space="PSUM") as adn:
                for g in range(KV):
                    for hh in range(GSZ):
                        h = g * GSZ + hh
                        P_h = pp.tile([128, NKT, 512], F16, tag="P")
                        for kc in range(NKT):
                            qb_lo, qb_hi = max(0, kc - 8), min(NQT - 1, kc)
                            qsl = slice(qb_lo * 128, (qb_hi + 1) * 128)
                            s_ps = asc.tile([128, 512], F32, tag="score")
                            nc.tensor.matmul(s_ps[:, qsl], kT[:, g, kc, :],
                                             qT[:, h, qsl], start=True, stop=True)
                            nc.scalar.activation(out=P_h[:, kc, qsl], in_=s_ps[:, qsl],
                                                 func=EXP, scale=rstdk[:, kc, g:g + 1])
                        # corner masks only: (qb, kc=qb) diag-causal and
                        # (qb, kc=qb+8) window edge, all 4 qb in one strided op.
                        # P_h free layout: kc*512 + q; diag tiles at qb*640,
                        # edge tiles at qb*640 + 8*512. masks: [p, qb, 2, 128].
                        pfull = P_h[:, 0, 0:128]
                        mfull = masks[:, 0, 0, :]
                        for jj, pbase in ((0, 0), (1, 8 * 512)):
                            pap = bass.AP(tensor=pfull.tensor,
                                          offset=pfull.offset + pbase,
                                          ap=[pfull.ap[0], [640, NQT], [1, 128]])
                            map_ = bass.AP(tensor=mfull.tensor,
                                           offset=mfull.offset + jj * 128,
                                           ap=[mfull.ap[0], [256, NQT], [1, 128]])
                            nc.vector.tensor_mul(pap, pap, map_)
                        # denominator: ones-matmul accumulation per query block
                        # (pad keys on interior tiles contribute exp(0)=1 each;
                        # corrected below via the host-computed padrow).
                        den_ps = adn.tile([1, 512], F32, tag="den")
                        for qb in range(NQT):
                            qbs = slice(qb * 128, (qb + 1) * 128)
                            for j in range(9):
                                nc.tensor.matmul(den_ps[0:1, qbs], ones_t,
                                                 P_h[:, qb + j, qbs],
                                                 start=(j == 0), stop=(j == 8))
                        o_ps = aoc.tile([128, 512], F32, tag="oacc")
                        for qb in range(NQT):
                            qbs = slice(qb * 128, (qb + 1) * 128)
                            for j in range(9):
                                nc.tensor.matmul(o_ps[:, qbs], vt[:, qb + j, g, :],
                                                 P_h[:, qb + j, qbs],
                                                 start=(j == 0), stop=(j == 8))
                        den_sb = asb.tile([1, 512], F32, tag="den_sb")
                        nc.vector.scalar_tensor_tensor(
                            out=den_sb, in0=den_ps[0:1, :], scalar=1.0,
                            in1=padrow_sb[0:1, :],
                            op0=mybir.AluOpType.mult,
                            op1=mybir.AluOpType.subtract)
                        den_rc = asb.tile([1, 512], F32, tag="den_rc")
                        nc.vector.reciprocal(out=den_rc, in_=den_sb)
                        recb = asb.tile([128, 512], F32, tag="recb")
                        nc.gpsimd.partition_broadcast(recb, den_rc[0:1, :])
                        nc.vector.tensor_mul(aoT[:, h, :], o_ps, recb)

            # ---------------- output projection ----------------
            with tc.tile_pool(name="o_w", bufs=2) as ow, \
                 tc.tile_pool(name="o_sb", bufs=3) as osb, \
                 tc.tile_pool(name="o_ps", bufs=3, space="PSUM") as ops:
                for ot in range(NDT):
                    wo_c = []
                    for i in range(4):
                        t = ow.tile([128, 4, 512], F16, tag=f"wo{i}")
                        nc.sync.dma_start(out=t, in_=wo_t[ot][:, 4 * i:4 * i + 4, :])
                        wo_c.append(t)
                    for sc in range(NQT):
                        ssl = slice(sc * 128, (sc + 1) * 128)
                        y_ps = ops.tile([128, 512], F32, tag="yacc")
                        for dc in range(NE):
                            nc.tensor.matmul(y_ps, aoT[:, dc, ssl],
                                             wo_c[dc // 4][:, dc % 4, :],
                                             start=(dc == 0), stop=(dc == NE - 1))
                        y_sb = osb.tile([128, 512], F32, tag="ysb")
                        nc.vector.tensor_copy(out=y_sb, in_=y_ps)
                        nc.sync.dma_start(
                            out=y[sc * 128:(sc + 1) * 128, ot * 512:(ot + 1) * 512],
                            in_=y_sb)

    nc.compile()
    return nc


# ---------------- host-side packing ----------------

def _tile_emajor(a16, col0, ncols):
    """[2048, N] (e-major) f16 array -> [128, 16, ncols] tiled view."""
    sl = a16[:, col0:col0 + ncols]
    return np.ascontiguousarray(sl.reshape(NE, 128, ncols).transpose(1, 0, 2))


def _rope_tables(pos, norm_w):
    """-> [128, 4, nchunks, 64] f32 tables (cosA, sinA, sinB, cosB) with the
    per-dim norm weights folded in. pos: [n*128] positions."""
    freqs = 1.0 / (THETA ** (np.arange(0, DK, 2, dtype=np.float64) / DK))
    ang = np.outer(pos.astype(np.float64), freqs)
    cos = np.cos(ang).astype(np.float32)
    sin = np.sin(ang).astype(np.float32)
    w_ev = norm_w[0::2].astype(np.float32)
    w_od = norm_w[1::2].astype(np.float32)
    tabs = np.stack([cos * w_ev, sin * w_od, sin * w_ev, cos * w_od])  # [4, n*128, 64]
    n = pos.shape[0] // 128
    return np.ascontiguousarray(
        tabs.reshape(4, n, 128, 64).transpose(2, 0, 1, 3))


def _masks_for_core(c):
    """Corner masks only: jj=0 -> kc=qb (window edge, lag 8);
    jj=1 -> kc=qb+8 (causal diagonal, lag 0)."""
    out = np.zeros((128, NQT, 2, 128), np.float16)
    p = np.arange(128)
    q = np.arange(128)
    for qb in range(NQT):
        for jj, j in ((0, 0), (1, 8)):
            kchunk = c * 4 - 8 + qb + j
            iglob = c * SQ + qb * 128 + q[None, :]
            jglob = kchunk * 128 + p[:, None]
            ok = (jglob >= 0) & (iglob - jglob >= 0) & (iglob - jglob < WINDOW)
            out[:, qb, jj, :] = ok.astype(np.float16)
    return out


def _padrow_for_core(c):
    """Per-q-position count of zero-padded keys that land on interior
    (unmasked) window tiles: exp(0)=1 each, subtracted from the softmax
    denominator. Interior tiles are kc=qb+1..qb+7; tile kc is fully padded
    iff global chunk c*4-8+qb+j < 0."""
    out = np.zeros((1, SQ), np.float32)
    for qb in range(NQT):
        npad = int(np.clip(7 - 4 * c - qb, 0, 7))
        out[0, qb * 128:(qb + 1) * 128] = 128.0 * npad
    return out


_PROGRAM = None


def _get_program():
    global _PROGRAM
    if _PROGRAM is None:
        _PROGRAM = build_program()
    return _PROGRAM


def _pack_in_maps(xq, xk, xv, Wq, Wk, Wv, Wo, q_norm_w, k_norm_w):
    xqT = np.ascontiguousarray(np.asarray(xq, np.float32)[0].T).astype(np.float16)
    xkT = np.asarray(xk, np.float32)[0].T.astype(np.float16)
    xvT = np.asarray(xv, np.float32)[0].T.astype(np.float16)
    pad = np.zeros((D, 2 * SQ), np.float16)
    xkTp = np.concatenate([pad, xkT], axis=1)  # col i = global row i - 1024
    xvTp = np.concatenate([pad, xvT], axis=1)

    wq16 = np.ascontiguousarray(np.asarray(Wq, np.float32).T).astype(np.float16)
    wk16 = np.ascontiguousarray(np.asarray(Wk, np.float32).T).astype(np.float16)
    wv16 = np.ascontiguousarray(np.asarray(Wv, np.float32).T).astype(np.float16)
    wo16 = np.ascontiguousarray(np.asarray(Wo, np.float32).T).astype(np.float16)

    wq_t = np.stack([_tile_emajor(wq16, dt * 512, 512) for dt in range(NDT)])
    wk_t = _tile_emajor(wk16, 0, 512)
    wv_t = _tile_emajor(wv16, 0, 512)
    wo_t = np.stack([_tile_emajor(wo16, ot * 512, 512) for ot in range(NDT)])

    qw = np.asarray(q_norm_w, np.float32)
    kw = np.asarray(k_norm_w, np.float32)

    in_maps = []
    for c in range(N_CORES):
        xq_t = _tile_emajor(xqT, c * SQ, SQ)
        xk_tc = np.stack([_tile_emajor(xkTp, (c + b) * 512, 512) for b in range(3)])
        xv_tc = np.stack([_tile_emajor(xvTp, (c + b) * 512, 512) for b in range(3)])
        qpos = c * SQ + np.arange(SQ)
        kpos = (c - 2) * 512 + np.arange(SKV)
        in_maps.append({
            "xq_t": xq_t, "xk_t": xk_tc, "xv_t": xv_tc,
            "wq_t": wq_t, "wk_t": wk_t, "wv_t": wv_t, "wo_t": wo_t,
            "ropeq": _rope_tables(qpos, qw),
            "ropek": _rope_tables(kpos, kw),
            "pmask": _masks_for_core(c),
            "padrow": _padrow_for_core(c),
        })
    return in_maps


def kernel(xq, xk, xv, Wq, Wk, Wv, Wo, q_norm_w, k_norm_w):
    nc = _get_program()
    in_maps = _pack_in_maps(xq, xk, xv, Wq, Wk, Wv, Wo, q_norm_w, k_norm_w)
    res = run_bass_kernel_spmd(nc, in_maps, core_ids=list(range(N_CORES)))
    out = np.concatenate([res.results[c]["y"] for c in range(N_CORES)], axis=0)
    return out.reshape(1, S, D).astype(np.float32)


def kernel_with_results(trace=False, tmpdir=None, **inputs):
    """Devloop entry: same as kernel() but also returns the raw
    BassKernelResults (exec_time_ns etc. when trace is enabled)."""
    nc = _get_program()
    in_maps = _pack_in_maps(**inputs)
    res = run_bass_kernel_spmd(nc, in_maps, core_ids=list(range(N_CORES)),
                               trace=trace, tmpdir=tmpdir)
    out = np.concatenate([res.results[c]["y"] for c in range(N_CORES)], axis=0)
    return out.reshape(1, S, D).astype(np.float32), res


# revision 24
# speedup vs baseline: 1.2395x; 1.0951x over previous
"""Trainium2 Bass kernel for nn_AttentionBlock (sliding-window GQA attention block).

Sharding: sequence-parallel over 8 cores. Core c owns query rows
[c*512, (c+1)*512) and recomputes K/V for the 3 aligned 512-row blocks
[(c-2)*512, (c+1)*512) that its 1024-wide causal window can touch
(out-of-range blocks are zero-padded and masked).

Per-core pipeline (all matmuls fp16 operands, fp32 PSUM accumulate):
  1. K/V projections -> RMS stats -> RoPE (norm weights folded into host
     RoPE tables; K's rstd folded into the softmax exp scale) -> PE
     transpose K to [dk, seq] layout; V kept [seq, dk].
  2. Q projection -> RMS/RoPE -> *rstd -> PE transpose to [dk, seq].
  3. Attention per head: scores computed transposed S^T[k, q] so that
     P^T tiles feed the PV matmul directly (lhsT = V). Softmax without
     max-subtraction (scores bounded ~5); denominator via ones-matmul;
     normalization applied to O^T with a gpsimd partition-broadcast of
     the reciprocal.
  4. Output projection from the transposed attention output, streamed
     against the (host-pre-transposed) Wo.
"""

import os
import sys

import numpy as np

for _p in ("/opt/trn_rl_repo",):
    if _p not in sys.path and os.path.isdir(_p):
        sys.path.insert(0, _p)

import concourse.bass as bass
import concourse.mybir as mybir
import concourse.tile as tile
from concourse import bacc
from concourse.bass_utils import run_bass_kernel_spmd
from concourse.masks import make_identity

F16 = mybir.dt.float16
F32 = mybir.dt.float32

N_CORES = 8
S, D = 4096, 2048
H, KV, DK = 16, 4, 128
GSZ = H // KV  # heads per kv group
WINDOW = 1024
THETA = 500000.0
EPS = 1e-6

SQ = S // N_CORES          # 512 query rows per core
NQT = SQ // 128            # 4 query chunks
NKT = 12                   # 12 kv chunks of 128 (3 blocks of 512)
SKV = NKT * 128            # 1536
NE = D // 128              # 16 contraction chunks
NDT = D // 512             # 4 tiles of 512 along output dims


def _broadcast_free(ap, count, axis):
    """Insert a 0-step (broadcast) free dim of length `count` at `axis`
    (free-dim index, 0-based after the partition dim)."""
    new = list(ap.ap)
    new.insert(1 + axis, [0, count])
    return bass.AP(tensor=ap.tensor, offset=ap.offset, ap=new)


def _rope_pairs(ap):
    """View a [128, n*128] AP as ([128, n, 64] even, [128, n, 64] odd)."""
    r = ap.rearrange("p (h m two) -> p h m two", two=2, m=64)
    return r[:, :, :, 0], r[:, :, :, 1]


def _emit_rope(nc, pool, src, dst, tabs, nheads, cast_scalars=None):
    """dst[:, h*128+d] = rope(src) using tables tabs = (cosA, sinA, sinB, cosB)
    each a [128, 64] AP broadcast across the nheads dim.

    If cast_scalars is given, it is a list of nheads [128,1] APs; the final
    per-head result is written as dst_head = tmp_head * scalar (fused cast).
    Otherwise results are written directly to dst.
    """
    ev, od = _rope_pairs(src)
    cosA, sinA, sinB, cosB = (_broadcast_free(t, nheads, 0) for t in tabs)
    if cast_scalars is None:
        out_ev, out_od = _rope_pairs(dst)
        tmp_ev, tmp_od = out_ev, out_od
        tmp = None
    else:
        tmp = pool.tile([128, nheads * 128], F32, tag="rope_tmp")
        tmp_ev, tmp_od = _rope_pairs(tmp)
    t1 = pool.tile([128, nheads, 64], F32, tag="rope_t1")
    t2 = pool.tile([128, nheads, 64], F32, tag="rope_t2")
    nc.vector.tensor_mul(t1, ev, cosA)
    nc.vector.tensor_mul(t2, od, sinA)
    nc.vector.tensor_sub(tmp_ev, t1, t2)
    t3 = pool.tile([128, nheads, 64], F32, tag="rope_t1")
    t4 = pool.tile([128, nheads, 64], F32, tag="rope_t2")
    nc.vector.tensor_mul(t3, ev, sinB)
    nc.vector.tensor_mul(t4, od, cosB)
    nc.vector.tensor_add(tmp_od, t3, t4)
    if cast_scalars is not None:
        for hh in range(nheads):
            nc.vector.tensor_scalar_mul(
                dst[:, hh * 128:(hh + 1) * 128],
                tmp[:, hh * 128:(hh + 1) * 128],
                cast_scalars[hh],
            )


def _rms_stats4(nc, pool, src, sqrt_bias, sqrt_scale, out_recip4):
    """out_recip4[128,4] = 1/sqrt(sum(head_sq)*sqrt_scale + sqrt_bias) for the
    four 128-wide head slices of a [128, 512] src tile."""
    ssq4 = pool.tile([128, 4], F32, tag="rms_ssq4")
    for hh in range(4):
        scr = pool.tile([128, 128], F32, tag="rms_scr")
        nc.scalar.activation(out=scr, in_=src[:, hh * 128:(hh + 1) * 128],
                             func=mybir.ActivationFunctionType.Square,
                             accum_out=ssq4[:, hh:hh + 1])
    srt4 = pool.tile([128, 4], F32, tag="rms_srt4")
    nc.scalar.activation(out=srt4, in_=ssq4, func=mybir.ActivationFunctionType.Sqrt,
                         bias=sqrt_bias, scale=sqrt_scale)
    nc.vector.reciprocal(out=out_recip4, in_=srt4)


def build_program():
    nc = bacc.Bacc("TRN2", target_bir_lowering=False, debug=False)

    xq_t = nc.declare_dram_parameter("xq_t", [128, NE, SQ], F16, isOutput=False)
    xk_t = nc.declare_dram_parameter("xk_t", [3, 128, NE, 512], F16, isOutput=False)
    xv_t = nc.declare_dram_parameter("xv_t", [3, 128, NE, 512], F16, isOutput=False)
    wq_t = nc.declare_dram_parameter("wq_t", [NDT, 128, NE, 512], F16, isOutput=False)
    wk_t = nc.declare_dram_parameter("wk_t", [128, NE, 512], F16, isOutput=False)
    wv_t = nc.declare_dram_parameter("wv_t", [128, NE, 512], F16, isOutput=False)
    wo_t = nc.declare_dram_parameter("wo_t", [NDT, 128, NE, 512], F16, isOutput=False)
    ropeq = nc.declare_dram_parameter("ropeq", [128, 4, NQT, 64], F32, isOutput=False)
    ropek = nc.declare_dram_parameter("ropek", [128, 4, NKT, 64], F32, isOutput=False)
    pmask = nc.declare_dram_parameter("pmask", [128, NQT, 2, 128], F16, isOutput=False)
    padrow = nc.declare_dram_parameter("padrow", [1, SQ], F32, isOutput=False)
    y = nc.declare_dram_parameter("y", [SQ, D], F32, isOutput=True)

    EXP = mybir.ActivationFunctionType.Exp

    with tile.TileContext(nc) as tc:
        with tc.tile_pool(name="const", bufs=1) as const, \
             tc.tile_pool(name="persist", bufs=1) as persist:
            ident = const.tile([128, 128], F16)
            make_identity(nc, ident)
            ones_t = const.tile([128, 1], F16)
            nc.vector.memset(ones_t, 1.0)
            bias_k = const.tile([128, 1], F32)
            nc.vector.memset(bias_k, 128.0 * EPS)
            bias_q = const.tile([128, 1], F32)
            nc.vector.memset(bias_q, EPS)
            # constants ride the (otherwise idle) GpSimd SWDGE ring so they
            # don't delay the K/V weight+activation loads on the Sync ring
            # at startup; rope-k first since the K epilogue needs it.
            rk_sb = const.tile([128, 4, NKT, 64], F32)
            nc.gpsimd.dma_start(out=rk_sb, in_=ropek[:, :, :, :])
            rq_sb = const.tile([128, 4, NQT, 64], F32)
            nc.gpsimd.dma_start(out=rq_sb, in_=ropeq[:, :, :, :])
            masks = const.tile([128, NQT, 2, 128], F16)
            nc.gpsimd.dma_start(out=masks, in_=pmask[:, :, :, :])
            padrow_sb = const.tile([1, SQ], F32)
            nc.gpsimd.dma_start(out=padrow_sb, in_=padrow[:, :])

            kT = persist.tile([128, KV, NKT, 128], F16)
            vt = persist.tile([128, NKT, KV, 128], F16)
            qT = persist.tile([128, H, SQ], F16)
            aoT = persist.tile([128, H, SQ], F16)
            rstdk = persist.tile([128, NKT, KV], F32)

            # ---------------- K/V phase ----------------
            # activations/weights staged as 4-ec-chunk tiles so the first
            # matmuls start after ~1MB rather than after the full 2MB load,
            # and chunk DMAs pipeline against the accumulation.
            with tc.tile_pool(name="kv_w", bufs=1) as kvw, \
                 tc.tile_pool(name="kv_stage", bufs=2) as kvs, \
                 tc.tile_pool(name="kv_sb", bufs=3) as kvsb, \
                 tc.tile_pool(name="kv_ps", bufs=3, space="PSUM") as kvps, \
                 tc.tile_pool(name="kv_tp", bufs=2, space="PSUM") as kvtp:
                # K-path chunks first on the Sync FIFO: the first projection
                # matmul only gates on xk chunk 0 + wk chunk 0 (~1MB).
                wk_c, wv_c = [], []
                xk_c, xv_c = [], []
                for i in range(4):
                    t = kvs.tile([128, 4, 512], F16, tag=f"xk{i}")
                    nc.sync.dma_start(out=t, in_=xk_t[0][:, 4 * i:4 * i + 4, :])
                    xk_c.append(t)
                    t = kvw.tile([128, 4, 512], F16, tag=f"wk{i}")
                    nc.sync.dma_start(out=t, in_=wk_t[:, 4 * i:4 * i + 4, :])
                    wk_c.append(t)
                for i in range(4):
                    t = kvs.tile([128, 4, 512], F16, tag=f"xv{i}")
                    nc.sync.dma_start(out=t, in_=xv_t[0][:, 4 * i:4 * i + 4, :])
                    xv_c.append(t)
                    t = kvw.tile([128, 4, 512], F16, tag=f"wv{i}")
                    nc.sync.dma_start(out=t, in_=wv_t[:, 4 * i:4 * i + 4, :])
                    wv_c.append(t)
                for b in range(3):
                    if b > 0:
                        xk_c, xv_c = [], []
                        for i in range(4):
                            t = kvs.tile([128, 4, 512], F16, tag=f"xk{i}")
                            nc.sync.dma_start(out=t, in_=xk_t[b][:, 4 * i:4 * i + 4, :])
                            xk_c.append(t)
                        for i in range(4):
                            t = kvs.tile([128, 4, 512], F16, tag=f"xv{i}")
                            nc.sync.dma_start(out=t, in_=xv_t[b][:, 4 * i:4 * i + 4, :])
                            xv_c.append(t)
                    for sc in range(4):
                        kc = b * 4 + sc
                        ssl = slice(sc * 128, (sc + 1) * 128)
                        k_ps = kvps.tile([128, 512], F32, tag="kps")
                        for ec in range(NE):
                            nc.tensor.matmul(k_ps, xk_c[ec // 4][:, ec % 4, ssl],
                                             wk_c[ec // 4][:, ec % 4, :],
                                             start=(ec == 0), stop=(ec == NE - 1))
                        _rms_stats4(nc, kvsb, k_ps, sqrt_bias=bias_k,
                                    sqrt_scale=1.0, out_recip4=rstdk[:, kc, :])
                        krot = kvsb.tile([128, 512], F16, tag="krot")
                        tabs = tuple(rk_sb[:, t, kc, :] for t in range(4))
                        _emit_rope(nc, kvsb, k_ps[:, :], krot[:, :], tabs, KV)
                        for g in range(KV):
                            ktp = kvtp.tile([128, 128], F16, tag="ktp")
                            nc.tensor.transpose(ktp, krot[:, g * 128:(g + 1) * 128], ident)
                            nc.vector.tensor_copy(out=kT[:, g, kc, :], in_=ktp)
                        v_ps = kvps.tile([128, 512], F32, tag="vps")
                        for ec in range(NE):
                            nc.tensor.matmul(v_ps, xv_c[ec // 4][:, ec % 4, ssl],
                                             wv_c[ec // 4][:, ec % 4, :],
                                             start=(ec == 0), stop=(ec == NE - 1))
                        nc.vector.tensor_copy(
                            out=vt[:, kc, :, :],
                            in_=v_ps.rearrange("p (g d) -> p g d", g=KV))

            # ---------------- Q phase ----------------
            with tc.tile_pool(name="q_stage", bufs=1) as qs, \
                 tc.tile_pool(name="q_w", bufs=2) as qw, \
                 tc.tile_pool(name="q_sb", bufs=3) as qsb, \
                 tc.tile_pool(name="q_ps", bufs=3, space="PSUM") as qps, \
                 tc.tile_pool(name="q_tps", bufs=2, space="PSUM") as qtps:
                xq_sb = qs.tile([128, NE, SQ], F16)
                nc.gpsimd.dma_start(out=xq_sb, in_=xq_t[:, :, :])
                for dt in range(NDT):
                    wq_c = []
                    for i in range(4):
                        t = qw.tile([128, 4, 512], F16, tag=f"wq{i}")
                        nc.sync.dma_start(out=t, in_=wq_t[dt][:, 4 * i:4 * i + 4, :])
                        wq_c.append(t)
                    for sc in range(NQT):
                        ssl = slice(sc * 128, (sc + 1) * 128)
                        q_ps = qps.tile([128, 512], F32, tag="qps")
                        for ec in range(NE):
                            nc.tensor.matmul(q_ps, xq_sb[:, ec, ssl],
                                             wq_c[ec // 4][:, ec % 4, :],
                                             start=(ec == 0), stop=(ec == NE - 1))
                        rq4 = qsb.tile([128, 4], F32, tag="rstdq4")
                        _rms_stats4(nc, qsb, q_ps, sqrt_bias=bias_q,
                                    sqrt_scale=1.0 / 128.0, out_recip4=rq4)
                        rstd_q = [rq4[:, hh:hh + 1] for hh in range(4)]
                        qrot = qsb.tile([128, 512], F16, tag="qrot")
                        tabs = tuple(rq_sb[:, t, sc, :] for t in range(4))
                        _emit_rope(nc, qsb, q_ps[:, :], qrot[:, :], tabs, 4,
                                   cast_scalars=rstd_q)
                        for hh in range(4):
                            h = dt * 4 + hh
                            qtp = qtps.tile([128, 128], F16, tag="qtp")
                            nc.tensor.transpose(qtp, qrot[:, hh * 128:(hh + 1) * 128],
                                                ident)
                            nc.vector.tensor_copy(out=qT[:, h, ssl], in_=qtp)

            # ---------------- attention phase ----------------
            import concourse.bass_isa as bass_isa
            with tc.tile_pool(name="p_pool", bufs=3) as pp, \
                 tc.tile_pool(name="a_sb", bufs=2) as asb, \
                 tc.tile_pool(name="a_sc", bufs=4, space="PSUM") as asc, \
                 tc.tile_pool(name="a_oc", bufs=2, space="PSUM") as aoc, \
                 tc.tile_pool(name="a_dn", bufs=2, space="PSUM") as adn:
                for g in range(KV):
                    for hh in range(GSZ):
                        h = g * GSZ + hh
                        P_h = pp.tile([128, NKT, 512], F16, tag="P")
                        for kc in range(NKT):
                            qb_lo, qb_hi = max(0, kc - 8), min(NQT - 1, kc)
                            qsl = slice(qb_lo * 128, (qb_hi + 1) * 128)
                            s_ps = asc.tile([128, 512], F32, tag="score")
                            nc.tensor.matmul(s_ps[:, qsl], kT[:, g, kc, :],
                                             qT[:, h, qsl], start=True, stop=True)
                            nc.scalar.activation(out=P_h[:, kc, qsl], in_=s_ps[:, qsl],
                                                 func=EXP, scale=rstdk[:, kc, g:g + 1])
                        # corner masks only: (qb, kc=qb) diag-causal and
                        # (qb, kc=qb+8) window edge, all 4 qb in one strided op.
                        # P_h free layout: kc*512 + q; diag tiles at qb*640,
                        # edge tiles at qb*640 + 8*512. masks: [p, qb, 2, 128].
                        pfull = P_h[:, 0, 0:128]
                        mfull = masks[:, 0, 0, :]
                        for jj, pbase in ((0, 0), (1, 8 * 512)):
                            pap = bass.AP(tensor=pfull.tensor,
                                          offset=pfull.offset + pbase,
                                          ap=[pfull.ap[0], [640, NQT], [1, 128]])
                            map_ = bass.AP(tensor=mfull.tensor,
                                           offset=mfull.offset + jj * 128,
                                           ap=[mfull.ap[0], [256, NQT], [1, 128]])
                            nc.vector.tensor_mul(pap, pap, map_)
                        # denominator: ones-matmul accumulation per query block
                        # (pad keys on interior tiles contribute exp(0)=1 each;
                        # corrected below via the host-computed padrow).
                        den_ps = adn.tile([1, 512], F32, tag="den")
                        for qb in range(NQT):
                            qbs = slice(qb * 128, (qb + 1) * 128)
                            for j in range(9):
                                nc.tensor.matmul(den_ps[0:1, qbs], ones_t,
                                                 P_h[:, qb + j, qbs],
                                                 start=(j == 0), stop=(j == 8))
                        o_ps = aoc.tile([128, 512], F32, tag="oacc")
                        for qb in range(NQT):
                            qbs = slice(qb * 128, (qb + 1) * 128)
                            for j in range(9):
                                nc.tensor.matmul(o_ps[:, qbs], vt[:, qb + j, g, :],
                                                 P_h[:, qb + j, qbs],
                                                 start=(j == 0), stop=(j == 8))
                        den_sb = asb.tile([1, 512], F32, tag="den_sb")
                        nc.vector.scalar_tensor_tensor(
                            out=den_sb, in0=den_ps[0:1, :], scalar=1.0,
                            in1=padrow_sb[0:1, :],
                            op0=mybir.AluOpType.mult,
                            op1=mybir.AluOpType.subtract)
                        den_rc = asb.tile([1, 512], F32, tag="den_rc")
                        nc.vector.reciprocal(out=den_rc, in_=den_sb)
                        recb = asb.tile([128, 512], F32, tag="recb")
                        nc.gpsimd.partition_broadcast(recb, den_rc[0:1, :])
                        nc.vector.tensor_mul(aoT[:, h, :], o_ps, recb)

            # ---------------- output projection ----------------
            with tc.tile_pool(name="o_w", bufs=2) as ow, \
                 tc.tile_pool(name="o_sb", bufs=3) as osb, \
                 tc.tile_pool(name="o_ps", bufs=3, space="PSUM") as ops:
                for ot in range(NDT):
                    wo_c = []
                    for i in range(4):
                        t = ow.tile([128, 4, 512], F16, tag=f"wo{i}")
                        nc.sync.dma_start(out=t, in_=wo_t[ot][:, 4 * i:4 * i + 4, :])
                        wo_c.append(t)
                    for sc in range(NQT):
                        ssl = slice(sc * 128, (sc + 1) * 128)
                        y_ps = ops.tile([128, 512], F32, tag="yacc")
                        for dc in range(NE):
                            nc.tensor.matmul(y_ps, aoT[:, dc, ssl],
                                             wo_c[dc // 4][:, dc % 4, :],
                                             start=(dc == 0), stop=(dc == NE - 1))
                        y_sb = osb.tile([128, 512], F32, tag="ysb")
                        nc.vector.tensor_copy(out=y_sb, in_=y_ps)
                        nc.sync.dma_start(
                            out=y[sc * 128:(sc + 1) * 128, ot * 512:(ot + 1) * 512],
                            in_=y_sb)

    nc.compile()
    return nc


# ---------------- host-side packing ----------------

def _tile_emajor(a16, col0, ncols):
    """[2048, N] (e-major) f16 array -> [128, 16, ncols] tiled view."""
    sl = a16[:, col0:col0 + ncols]
    return np.ascontiguousarray(sl.reshape(NE, 128, ncols).transpose(1, 0, 2))


def _rope_tables(pos, norm_w):
    """-> [128, 4, nchunks, 64] f32 tables (cosA, sinA, sinB, cosB) with the
    per-dim norm weights folded in. pos: [n*128] positions."""
    freqs = 1.0 / (THETA ** (np.arange(0, DK, 2, dtype=np.float64) / DK))
    ang = np.outer(pos.astype(np.float64), freqs)
    cos = np.cos(ang).astype(np.float32)
    sin = np.sin(ang).astype(np.float32)
    w_ev = norm_w[0::2].astype(np.float32)
    w_od = norm_w[1::2].astype(np.float32)
    tabs = np.stack([cos * w_ev, sin * w_od, sin * w_ev, cos * w_od])  # [4, n*128, 64]
    n = pos.shape[0] // 128
    return np.ascontiguousarray(
        tabs.reshape(4, n, 128, 64).transpose(2, 0, 1, 3))


def _masks_for_core(c):
    """Corner masks only: jj=0 -> kc=qb (window edge, lag 8);
    jj=1 -> kc=qb+8 (causal diagonal, lag 0)."""
    out = np.zeros((128, NQT, 2, 128), np.float16)
    p = np.arange(128)
    q = np.arange(128)
    for qb in range(NQT):
        for jj, j in ((0, 0), (1, 8)):
            kchunk = c * 4 - 8 + qb + j
            iglob = c * SQ + qb * 128 + q[None, :]
            jglob = kchunk * 128 + p[:, None]
            ok = (jglob >= 0) & (iglob - jglob >= 0) & (iglob - jglob < WINDOW)
            out[:, qb, jj, :] = ok.astype(np.float16)
    return out


def _padrow_for_core(c):
    """Per-q-position count of zero-padded keys that land on interior
    (unmasked) window tiles: exp(0)=1 each, subtracted from the softmax
    denominator. Interior tiles are kc=qb+1..qb+7; tile kc is fully padded
    iff global chunk c*4-8+qb+j < 0."""
    out = np.zeros((1, SQ), np.float32)
    for qb in range(NQT):
        npad = int(np.clip(7 - 4 * c - qb, 0, 7))
        out[0, qb * 128:(qb + 1) * 128] = 128.0 * npad
    return out


_PROGRAM = None


def _get_program():
    global _PROGRAM
    if _PROGRAM is None:
        _PROGRAM = build_program()
    return _PROGRAM


def _pack_in_maps(xq, xk, xv, Wq, Wk, Wv, Wo, q_norm_w, k_norm_w):
    xqT = np.ascontiguousarray(np.asarray(xq, np.float32)[0].T).astype(np.float16)
    xkT = np.asarray(xk, np.float32)[0].T.astype(np.float16)
    xvT = np.asarray(xv, np.float32)[0].T.astype(np.float16)
    pad = np.zeros((D, 2 * SQ), np.float16)
    xkTp = np.concatenate([pad, xkT], axis=1)  # col i = global row i - 1024
    xvTp = np.concatenate([pad, xvT], axis=1)

    wq16 = np.ascontiguousarray(np.asarray(Wq, np.float32).T).astype(np.float16)
    wk16 = np.ascontiguousarray(np.asarray(Wk, np.float32).T).astype(np.float16)
    wv16 = np.ascontiguousarray(np.asarray(Wv, np.float32).T).astype(np.float16)
    wo16 = np.ascontiguousarray(np.asarray(Wo, np.float32).T).astype(np.float16)

    wq_t = np.stack([_tile_emajor(wq16, dt * 512, 512) for dt in range(NDT)])
    wk_t = _tile_emajor(wk16, 0, 512)
    wv_t = _tile_emajor(wv16, 0, 512)
    wo_t = np.stack([_tile_emajor(wo16, ot * 512, 512) for ot in range(NDT)])

    qw = np.asarray(q_norm_w, np.float32)
    kw = np.asarray(k_norm_w, np.float32)

    in_maps = []
    for c in range(N_CORES):
        xq_t = _tile_emajor(xqT, c * SQ, SQ)
        xk_tc = np.stack([_tile_emajor(xkTp, (c + b) * 512, 512) for b in range(3)])
        xv_tc = np.stack([_tile_emajor(xvTp, (c + b) * 512, 512) for b in range(3)])
        qpos = c * SQ + np.arange(SQ)
        kpos = (c - 2) * 512 + np.arange(SKV)
        in_maps.append({
            "xq_t": xq_t, "xk_t": xk_tc, "xv_t": xv_tc,
            "wq_t": wq_t, "wk_t": wk_t, "wv_t": wv_t, "wo_t": wo_t,
            "ropeq": _rope_tables(qpos, qw),
            "ropek": _rope_tables(kpos, kw),
            "pmask": _masks_for_core(c),
            "padrow": _padrow_for_core(c),
        })
    return in_maps


def kernel(xq, xk, xv, Wq, Wk, Wv, Wo, q_norm_w, k_norm_w):
    nc = _get_program()
    in_maps = _pack_in_maps(xq, xk, xv, Wq, Wk, Wv, Wo, q_norm_w, k_norm_w)
    res = run_bass_kernel_spmd(nc, in_maps, core_ids=list(range(N_CORES)))
    out = np.concatenate([res.results[c]["y"] for c in range(N_CORES)], axis=0)
    return out.reshape(1, S, D).astype(np.float32)


def kernel_with_results(trace=False, tmpdir=None, **inputs):
    """Devloop entry: same as kernel() but also returns the raw
    BassKernelResults (exec_time_ns etc. when trace is enabled)."""
    nc = _get_program()
    in_maps = _pack_in_maps(**inputs)
    res = run_bass_kernel_spmd(nc, in_maps, core_ids=list(range(N_CORES)),
                               trace=trace, tmpdir=tmpdir)
    out = np.concatenate([res.results[c]["y"] for c in range(N_CORES)], axis=0)
    return out.reshape(1, S, D).astype(np.float32), res
